# revision 1
# baseline (speedup 1.0000x reference)
"""Data-parallel ATTNRNNAgent kernel for 8 trn2 NeuronCores.

Shards the batch axis (4096) of inputs/masks/hidden_state across the 8
cores (512 each) and replicates the tiny weights, per the sharding hint.
The per-core computation (fc1+ReLU -> masked multi-head attention ->
GRU cell -> output head) is jit-compiled for the Neuron cores via pmap.
"""

import functools

import jax
import jax.numpy as jnp
import numpy as np

N_AGENTS = 32
N_HEADS = 8
HEAD_DIM = 32
HID = 256
ATT = 256
N_ACT = 32
NEG = -1e9
M = 8  # cores


def _forward(inputs, hidden_state, obs_mask, scenario_mask, fc1_w, fc1_b,
             q_w, q_b, k_w, k_b, v_w, v_b, w_ih, w_hh, b_ih, b_hh,
             out_w, out_b):
    B, N, _ = inputs.shape
    A, nh, e = N_AGENTS, N_HEADS, HEAD_DIM
    x = jax.nn.relu(inputs @ fc1_w.T + fc1_b)
    q = (x[:, :A] @ q_w.T + q_b).reshape(B, A, nh, e).transpose(0, 2, 1, 3)
    k = (x @ k_w.T + k_b).reshape(B, N, nh, e).transpose(0, 2, 3, 1)
    v = jax.nn.relu(x @ v_w.T + v_b).reshape(B, N, nh, e).transpose(0, 2, 1, 3)
    score = jnp.einsum('bhae,bhen->bhan', q, k) / np.float32(np.sqrt(e))
    m = obs_mask[:, :A, :][:, None, :, :]
    score = jnp.where(m, NEG, score)
    w = jax.nn.softmax(score, axis=-1)
    w = jnp.where(m, 0.0, w)
    att = jnp.einsum('bhan,bhne->bhae', w, v)
    att = att.transpose(0, 2, 1, 3).reshape(B * A, nh * e)
    h0 = hidden_state.reshape(B * A, HID)
    gi = att @ w_ih.T + b_ih
    gh = h0 @ w_hh.T + b_hh
    ir, iz, in_ = jnp.split(gi, 3, axis=-1)
    hr, hz, hn = jnp.split(gh, 3, axis=-1)
    r = jax.nn.sigmoid(ir + hr)
    z = jax.nn.sigmoid(iz + hz)
    n = jnp.tanh(in_ + r * hn)
    h = (1.0 - z) * n + z * h0
    h = h.reshape(B, A, HID)
    am = scenario_mask[:, :, None]
    h = jnp.where(am, 0.0, h)
    q_out = h @ out_w.T + out_b
    q_out = jnp.where(am, 0.0, q_out)
    return q_out, h


@functools.partial(
    jax.pmap,
    in_axes=(0, 0, 0, 0) + (None,) * 14,
    axis_name='cores',
)
def _pmapped(inputs, hidden_state, obs_mask, scenario_mask, *weights):
    return _forward(inputs, hidden_state, obs_mask, scenario_mask, *weights)


def kernel(inputs, hidden_state, fc1_w, fc1_b, q_w, q_b, k_w, k_b, v_w, v_b,
           w_ih, w_hh, b_ih, b_hh, out_w, out_b, obs_mask, scenario_mask):
    B = inputs.shape[0]
    Bs = B // M
    shard = lambda t: np.asarray(t).reshape((M, Bs) + t.shape[1:])
    q_out, h = _pmapped(
        shard(inputs), shard(hidden_state), shard(obs_mask),
        shard(scenario_mask),
        fc1_w, fc1_b, q_w, q_b, k_w, k_b, v_w, v_b,
        w_ih, w_hh, b_ih, b_hh, out_w, out_b)
    q_out = np.asarray(q_out).reshape(B, N_AGENTS, N_ACT)
    h = np.asarray(h).reshape(B, N_AGENTS, HID)
    return q_out, h
